# revision 52
# baseline (speedup 1.0000x reference)
"""GQA attention block (B=2, T=2048, D=2048, 16 Q heads, 4 KV heads, RoPE,
causal, out-projection) on 8 Trainium2 NeuronCores.

Sharding: core i = (batch b = i//4, kv-group g = i%4). Each core computes the
4 query heads of its kv-group for its batch, then a partial output projection
with the matching 512 rows of wo; the host sums the 4 partials per batch.

v2 (fp16 datapath, PE kept saturated):
  - All matmul operands fp16 (1 cycle/row at any free width, half the DMA);
    PSUM accumulation stays fp32. The 1/sqrt(d) score scale is folded into
    the exp activation's scale, so weights ship unscaled.
  - Phase 1 (per t-chunk): swap-matmul projections (x^T chunk stationary,
    [wq|wk|wv] moving), RoPE on natural-layout PSUM (q halves on DVE, k
    halves on Pool), fp16 PE transposes to Q^T/K^T.
  - Phase 2 (per h, tb): S^T = K^T.T @ Q^T trimmed to the causal 128-col
    granularity; exp on ACT (scale=1/sqrt d) straight to fp16; diagonal
    blocks masked AFTER exp by a 0/1 triangle multiply on DVE; denominator
    accumulated as fp16 chunk sums on DVE, partition-reduced by a ones-column
    matmul, reciprocal via ACT exp(-ln), broadcast back by a ones-row matmul,
    final normalize multiply on DVE.
  - Phase 3 (out-projection) is interleaved into the NEXT tb's attention as
    PE filler work so the tensor engine never idles (keeps the 2.4 GHz
    p-state); stage copies run on Pool; fp16 partials DMA out per t-chunk.
"""

import math

import numpy as np

import concourse.bass as bass
import concourse.bacc as bacc
import concourse.mybir as mybir
from concourse.bass_utils import run_bass_kernel_spmd
from concourse.masks import make_identity
from concourse.tile import TileContext

F32 = mybir.dt.float32
F16 = mybir.dt.float16
AF = mybir.ActivationFunctionType

D_MODEL = 2048
T = 2048
B = 2
N_HEADS = 16
N_KV = 4
HEAD_DIM = 128
GH = N_HEADS // N_KV  # 4 q heads per core
HALF = HEAD_DIM // 2
KD = D_MODEL // 128   # 16 contraction chunks
TC = T // 128         # 16 t-chunks of 128
TB = T // 512         # 4 t-chunks of 512
SCALE = 1.0 / math.sqrt(HEAD_DIM)


def build_nc() -> bass.Bass:
    nc = bacc.Bacc("TRN2", target_bir_lowering=False)

    xt = nc.declare_dram_parameter("xt", [TC, 128, KD, 128], F16, isOutput=False)
    w = nc.declare_dram_parameter("w", [128, KD, 768], F16, isOutput=False)
    wo = nc.declare_dram_parameter("wo", [128, GH, D_MODEL], F16, isOutput=False)
    cs = nc.declare_dram_parameter("cs", [128, TC, 128], F32, isOutput=False)
    tri = nc.declare_dram_parameter("tri", [128, 128], F16, isOutput=False)
    out = nc.declare_dram_parameter("out", [T, D_MODEL], F16, isOutput=True)

    with TileContext(nc) as tc:
        with tc.tile_pool(name="persist", bufs=1) as persist:
            W = persist.tile([128, KD, 768], F16)
            WO = persist.tile([128, GH, D_MODEL], F16)
            CS = persist.tile([128, TC, 128], F32)
            identF = persist.tile([128, 128], F16)
            TRI = persist.tile([128, 128], F16)
            ONESC = persist.tile([128, 1], F16)
            ONESR = persist.tile([1, 128], F16)
            QTs = [[persist.tile([128, 512], F16, name=f"qt{h}_{tb}")
                    for tb in range(TB)] for h in range(GH)]
            KTs = [persist.tile([128, 128], F16, name=f"kt{s}")
                   for s in range(TC)]
            Vs = [persist.tile([128, 128], F16, name=f"v{s}")
                  for s in range(TC)]
            OTs = [[persist.tile([128, 512], F16, name=f"ot{h}_{tb}")
                    for tb in range(TB)] for h in range(GH)]

            make_identity(nc, identF)
            # TRI[s, c] = 1 where c >= s (keep), else 0: causal keep-mask for
            # the diagonal 128x128 block.
            nc.gpsimd.memset(ONESC, 1.0)
            nc.gpsimd.memset(ONESR, 1.0)



            # ---- phase 1: projections + rope + transposes -------------
            with (
                tc.tile_pool(name="xtp", bufs=2) as xtp,
                tc.tile_pool(name="ropedst", bufs=2) as ropedst,
                tc.tile_pool(name="ropetmp", bufs=3) as ropetmp,
                tc.tile_pool(name="pq", bufs=2, space="PSUM") as pqp,
                tc.tile_pool(name="pkv", bufs=2, space="PSUM") as pkvp,
                tc.tile_pool(name="ptr", bufs=2, space="PSUM") as ptrp,
            ):
                def wchunk(c):
                    nc.scalar.dma_start(
                        out=W[:, c * 4:(c + 1) * 4, :]
                        .rearrange("p k c -> p (k c)"),
                        in_=w[:, c * 4:(c + 1) * 4, :]
                        .rearrange("p k c -> p (k c)"))

                dsts = {}

                def emit_transpose(t, j):
                    # transpose roped q heads + k into QT / KT (fp16)
                    tp = ptrp.tile([128, 128], F16, tag="tp",
                                   padded_shape=[128, 1024],
                                   name=f"tp{t}_{j}")
                    nc.tensor.transpose(tp, dsts[t][:, j * 128:(j + 1) * 128],
                                        identF)
                    if j < GH:
                        nc.scalar.copy(
                            QTs[j][t // 4][:, (t % 4) * 128:(t % 4 + 1) * 128],
                            tp)
                    else:
                        nc.scalar.copy(KTs[t], tp)

                for t in range(TC):
                    xt_t = xtp.tile([128, KD, 128], F16)
                    if t == 0:
                        # interleave the first x^T chunk with the constants so
                        # the k-th projection's inputs land just in time
                        for c in range(4):
                            nc.sync.dma_start(
                                out=xt_t[:, c * 4:(c + 1) * 4, :],
                                in_=xt[0][:, c * 4:(c + 1) * 4, :]
                                .rearrange("p k c -> p (k c)"))
                            wchunk(c)
                            if c == 0:
                                nc.scalar.dma_start(
                                    out=CS, in_=cs.rearrange("p k c -> p (k c)"))
                        nc.scalar.dma_start(out=TRI, in_=tri[:, :])
                    else:
                        nc.sync.dma_start(out=xt_t,
                                          in_=xt[t].rearrange("p k c -> p (k c)"))
                    pq = pqp.tile([128, 512], F32)
                    pkv = pkvp.tile([128, 256], F32, padded_shape=[128, 512])
                    for k in range(KD):
                        lhs = xt_t[:, k, :]
                        nc.tensor.matmul(pq, lhs, W[:, k, 0:512],
                                         start=(k == 0), stop=(k == KD - 1))
                        nc.tensor.matmul(pkv, lhs, W[:, k, 512:768],
                                         start=(k == 0), stop=(k == KD - 1))
                        if t > 0 and 4 <= k < 9:
                            emit_transpose(t - 1, k - 4)
                    # rope (q: 4 heads batched as 3D on DVE; k: single head on Pool)
                    dst = ropedst.tile([128, 640], F16)
                    dst3 = dst.rearrange("p (h c) -> p h c", c=128)
                    pq3 = pq.rearrange("p (h c) -> p h c", c=128)
                    cosb = CS[:, t, None, 0:HALF].to_broadcast((128, GH, HALF))
                    sinb = CS[:, t, None, HALF:128].to_broadcast((128, GH, HALF))
                    q1, q2 = pq3[:, :, 0:HALF], pq3[:, :, HALF:128]
                    t1 = ropetmp.tile([128, GH, HALF], F32, tag="rt")
                    t2 = ropetmp.tile([128, GH, HALF], F32, tag="rt")
                    nc.vector.tensor_mul(t1, q1, cosb)
                    nc.vector.tensor_mul(t2, q2, sinb)
                    nc.vector.tensor_sub(dst3[:, 0:GH, 0:HALF], t1, t2)
                    t3 = ropetmp.tile([128, GH, HALF], F32, tag="rt")
                    t4 = ropetmp.tile([128, GH, HALF], F32, tag="rt")
                    nc.vector.tensor_mul(t3, q2, cosb)
                    nc.vector.tensor_mul(t4, q1, sinb)
                    nc.vector.tensor_add(dst3[:, 0:GH, HALF:128], t3, t4)
                    cos2, sin2 = CS[:, t, 0:HALF], CS[:, t, HALF:128]
                    k1, k2 = pkv[:, 0:HALF], pkv[:, HALF:128]
                    t5 = ropetmp.tile([128, HALF], F32, tag="rk")
                    t6 = ropetmp.tile([128, HALF], F32, tag="rk")
                    nc.vector.tensor_mul(t5, k1, cos2)
                    nc.vector.tensor_mul(t6, k2, sin2)
                    nc.vector.tensor_sub(dst[:, 512:576], t5, t6)
                    t7 = ropetmp.tile([128, HALF], F32, tag="rk")
                    t8 = ropetmp.tile([128, HALF], F32, tag="rk")
                    nc.vector.tensor_mul(t7, k2, cos2)
                    nc.vector.tensor_mul(t8, k1, sin2)
                    nc.vector.tensor_add(dst[:, 576:640], t7, t8)
                    nc.scalar.copy(Vs[t], pkv[:, 128:256])
                    dsts[t] = dst
                for j in range(5):
                    emit_transpose(TC - 1, j)

            # wo arrives while attention runs.
            nc.scalar.dma_start(out=WO, in_=wo.rearrange("p h c -> p (h c)"))

            # ---- phase 2+3: attention with interleaved out-projection --
            with (
                tc.tile_pool(name="ptp", bufs=6) as ptp,
                tc.tile_pool(name="laccp", bufs=2) as laccp,
                tc.tile_pool(name="recp", bufs=2) as recp,
                tc.tile_pool(name="stagep", bufs=2) as stagep,
                tc.tile_pool(name="pst", bufs=2, space="PSUM") as pstp,
                tc.tile_pool(name="pot", bufs=2, space="PSUM") as potp,
                tc.tile_pool(name="pden", bufs=1, space="PSUM") as pdenp,
                tc.tile_pool(name="pbc", bufs=1, space="PSUM") as pbcp,
                tc.tile_pool(name="po", bufs=2, space="PSUM") as pop,
            ):
                def outproj_units(tb):
                    """Yield one emission step of out-projection for block tb."""
                    for i in range(4):
                        tq = tb * 4 + i
                        stage = stagep.tile([128, D_MODEL], F16, tag="stage",
                                            name=f"stage{tq}")
                        for n in range(4):
                            po = pop.tile([128, 512], F32, tag="po",
                                          name=f"po{tq}_{n}")
                            for h in range(GH):
                                yield (nc.tensor.matmul, po,
                                       OTs[h][tb][:, i * 128:(i + 1) * 128],
                                       WO[:, h, n * 512:(n + 1) * 512],
                                       dict(start=(h == 0), stop=(h == GH - 1)))
                            yield (nc.scalar.copy,
                                   stage[:, n * 512:(n + 1) * 512], po)
                        yield (nc.scalar.dma_start,
                               dict(out=out[tq * 128:(tq + 1) * 128, :],
                                    in_=stage))

                def run_unit(u):
                    if isinstance(u[-1], dict) and len(u) == 2:
                        u[0](**u[1])
                    elif isinstance(u[-1], dict):
                        u[0](*u[1:-1], **u[-1])
                    else:
                        u[0](*u[1:])

                filler = None

                def fill(k):
                    nonlocal filler
                    if filler is None:
                        return
                    for _ in range(k):
                        u = next(filler, None)
                        if u is None:
                            filler = None
                            return
                        run_unit(u)

                pending = None
                pending_cross = False
                for tb in range(TB):
                    for h in range(GH):
                        nsc = 4 * (tb + 1)
                        # the tb-crossing normalize must land before the filler
                        # that reads OTs[3][tb-1] (consumed at sc==7)
                        flush_sc = min(5 if pending_cross else 7, nsc - 1)
                        ot = potp.tile([128, 512], F32, tag="ot",
                                       name=f"otp{h}_{tb}")
                        lacc = laccp.tile([128, 512], F16, tag="lacc",
                                          name=f"la{h}_{tb}")
                        for sc in range(nsc):
                            off = max(0, sc - 4 * tb) * 128
                            wdt = 512 - off
                            st = pstp.tile([128, 512], F32, tag="st",
                                           name=f"st{h}_{tb}_{sc}")
                            nc.tensor.matmul(st[:, 0:wdt], KTs[sc],
                                             QTs[h][tb][:, off:512],
                                             start=True, stop=True)
                            if sc == flush_sc and pending is not None:
                                pending()
                                pending = None
                            if sc >= 4:
                                fill(1)
                            pt = ptp.tile([128, 512], F16, tag="pt",
                                          name=f"pt{h}_{tb}_{sc}")
                            nc.scalar.activation(pt[:, 0:wdt], st[:, 0:wdt],
                                                 AF.Exp, scale=SCALE)
                            if sc >= 4 * tb:
                                nc.vector.tensor_mul(pt[:, 0:128],
                                                     pt[:, 0:128], TRI)
                            if sc == 0:
                                nc.vector.tensor_copy(lacc, pt)
                            else:
                                nc.vector.tensor_add(lacc[:, off:512],
                                                     lacc[:, off:512],
                                                     pt[:, 0:wdt])
                            nc.tensor.matmul(ot[:, off:512], Vs[sc],
                                             pt[:, 0:wdt],
                                             start=(sc == 0),
                                             stop=(sc == nsc - 1))
                        # keep the PE fed while DVE finishes the block's
                        # denominator adds
                        fill(4)
                        # transposed denominator: denT[q,0] = sum_s lacc[s,q]
                        # per q-chunk (~1-cycle matmuls), so the reciprocal
                        # runs on 128 partitions (~0.1us instead of 3.3us)
                        denT = pdenp.tile([128, 4], F32, tag="den",
                                          name=f"den{h}_{tb}")
                        for qc in range(4):
                            nc.tensor.matmul(denT[:, qc:qc + 1],
                                             lacc[:, qc * 128:(qc + 1) * 128],
                                             ONESC, start=True, stop=True)
                        recT = recp.tile([128, 4], F16, tag="recT",
                                         name=f"recT{h}_{tb}")
                        with nc.allow_low_precision("fp16 softmax denominators"):
                            nc.vector.reciprocal(recT, denT)
                        # scatter the reciprocal column back into row layout
                        # with a tiny SBUF->SBUF DMA (off both PE and DVE)
                        # linear scatter: rec stores q interleaved as p*4+c;
                        # the bc matmul's rhs AP undoes the permutation
                        rec = recp.tile([1, 512], F16, tag="rec",
                                        name=f"rec{h}_{tb}")
                        nc.sync.dma_start(out=rec, in_=recT)
                        # unnormalized copy frees the ot PSUM bank early
                        nc.vector.tensor_copy(OTs[h][tb], ot)

                        def fin(h=h, tb=tb, rec=rec):
                            bc = pbcp.tile([128, 512], F32, tag="bc",
                                           name=f"bc{h}_{tb}")
                            nc.tensor.matmul(
                                bc, ONESR,
                                rec.rearrange("a (p c) -> a c p", c=4),
                                start=True, stop=True)
                            nc.vector.tensor_mul(OTs[h][tb], OTs[h][tb], bc)
                        pending = fin
                        pending_cross = (h == GH - 1)
                        fill(2)
                    # drain leftover fillers of the previous block before
                    # switching to this block's out-projection
                    fill(10**6)
                    filler = outproj_units(tb)
                    if tb == TB - 1 and pending is not None:
                        fill(3)
                        pending()
                        pending = None
                # tail: the last block's out-projection
                fill(10**6)

    nc.compile()
    return nc


def _prep_core_inputs(x_b, wq, wk, wv, wo, cs_cat, g):
    wq_g = wq[:, g * 512:(g + 1) * 512]
    wk_g = wk[:, g * 128:(g + 1) * 128]
    wv_g = wv[:, g * 128:(g + 1) * 128]
    wqkv = np.concatenate([wq_g, wk_g, wv_g], axis=1)          # [D, 768]
    w_t = np.ascontiguousarray(wqkv.reshape(KD, 128, 768).transpose(1, 0, 2))
    wo_g = wo[g * 512:(g + 1) * 512, :]                         # [512, D]
    wo_t = np.ascontiguousarray(wo_g.reshape(GH, 128, D_MODEL).transpose(1, 0, 2))
    xt = np.ascontiguousarray(
        x_b.reshape(TC, 128, KD, 128).transpose(0, 3, 2, 1))    # [tc,j,k,i]
    tri = (np.arange(128)[None, :] >= np.arange(128)[:, None])
    return {
        "xt": xt.astype(np.float16),
        "w": w_t.astype(np.float16),
        "wo": wo_t.astype(np.float16),
        "cs": cs_cat.astype(np.float32),
        "tri": tri.astype(np.float16),
    }


def _prep_all(x, wq, wk, wv, wo, cos, sin):
    cs = np.concatenate([cos, sin], axis=1)                     # [T, 128]
    cs_t = np.ascontiguousarray(
        cs.reshape(TC, 128, 128).transpose(1, 0, 2)).astype(np.float32)
    in_maps = []
    for i in range(8):
        b, g = i // 4, i % 4
        in_maps.append(_prep_core_inputs(x[b], wq, wk, wv, wo, cs_t, g))
    return in_maps


def kernel(x, wq, wk, wv, wo, cos, sin):
    x = np.asarray(x, np.float32)
    wq = np.asarray(wq, np.float32)
    wk = np.asarray(wk, np.float32)
    wv = np.asarray(wv, np.float32)
    wo = np.asarray(wo, np.float32)
    cos = np.asarray(cos, np.float32)
    sin = np.asarray(sin, np.float32)

    nc = build_nc()
    in_maps = _prep_all(x, wq, wk, wv, wo, cos, sin)
    res = run_bass_kernel_spmd(nc, in_maps, list(range(8)))
    outs = [np.asarray(res.results[i]["out"], np.float32) for i in range(8)]
    full = np.empty((B, T, D_MODEL), np.float32)
    for b in range(B):
        full[b] = outs[4 * b] + outs[4 * b + 1] + outs[4 * b + 2] + outs[4 * b + 3]
    return full


# revision 53
# speedup vs baseline: 1.0448x; 1.0448x over previous
"""GQA attention block (B=2, T=2048, D=2048, 16 Q heads, 4 KV heads, RoPE,
causal, out-projection) on 8 Trainium2 NeuronCores.

Sharding: core i = (batch b = i//4, kv-group g = i%4). Each core computes the
4 query heads of its kv-group for its batch, then a partial output projection
with the matching 512 rows of wo; the host sums the 4 partials per batch.

v2 (fp16 datapath, PE kept saturated):
  - All matmul operands fp16 (1 cycle/row at any free width, half the DMA);
    PSUM accumulation stays fp32. The 1/sqrt(d) score scale is folded into
    the exp activation's scale, so weights ship unscaled.
  - Phase 1 (per t-chunk): swap-matmul projections (x^T chunk stationary,
    [wq|wk|wv] moving), RoPE on natural-layout PSUM (q halves on DVE, k
    halves on Pool), fp16 PE transposes to Q^T/K^T.
  - Phase 2 (per h, tb): S^T = K^T.T @ Q^T trimmed to the causal 128-col
    granularity; exp on ACT (scale=1/sqrt d) straight to fp16; diagonal
    blocks masked AFTER exp by a 0/1 triangle multiply on DVE; denominator
    accumulated as fp16 chunk sums on DVE, partition-reduced by a ones-column
    matmul, reciprocal via ACT exp(-ln), broadcast back by a ones-row matmul,
    final normalize multiply on DVE.
  - Phase 3 (out-projection) is interleaved into the NEXT tb's attention as
    PE filler work so the tensor engine never idles (keeps the 2.4 GHz
    p-state); stage copies run on Pool; fp16 partials DMA out per t-chunk.
"""

import math

import numpy as np

import concourse.bass as bass
import concourse.bacc as bacc
import concourse.mybir as mybir
from concourse.bass_utils import run_bass_kernel_spmd
from concourse.masks import make_identity
from concourse.tile import TileContext

F32 = mybir.dt.float32
F16 = mybir.dt.float16
AF = mybir.ActivationFunctionType

D_MODEL = 2048
T = 2048
B = 2
N_HEADS = 16
N_KV = 4
HEAD_DIM = 128
GH = N_HEADS // N_KV  # 4 q heads per core
HALF = HEAD_DIM // 2
KD = D_MODEL // 128   # 16 contraction chunks
TC = T // 128         # 16 t-chunks of 128
TB = T // 512         # 4 t-chunks of 512
SCALE = 1.0 / math.sqrt(HEAD_DIM)


def build_nc() -> bass.Bass:
    nc = bacc.Bacc("TRN2", target_bir_lowering=False)

    xt = nc.declare_dram_parameter("xt", [TC, 128, KD, 128], F16, isOutput=False)
    w = nc.declare_dram_parameter("w", [128, KD, 768], F16, isOutput=False)
    wo = nc.declare_dram_parameter("wo", [128, GH, D_MODEL], F16, isOutput=False)
    cs = nc.declare_dram_parameter("cs", [128, TC, 128], F32, isOutput=False)
    tri = nc.declare_dram_parameter("tri", [128, 128], F16, isOutput=False)
    out = nc.declare_dram_parameter("out", [T, D_MODEL], F16, isOutput=True)

    with TileContext(nc) as tc:
        with tc.tile_pool(name="persist", bufs=1) as persist:
            W = persist.tile([128, KD, 768], F16)
            WO = persist.tile([128, GH, D_MODEL], F16)
            CS = persist.tile([128, TC, 128], F32)
            identF = persist.tile([128, 128], F16)
            TRI = persist.tile([128, 128], F16)
            ONESC = persist.tile([128, 1], F16)
            ONESR = persist.tile([1, 128], F16)
            QTs = [[persist.tile([128, 512], F16, name=f"qt{h}_{tb}")
                    for tb in range(TB)] for h in range(GH)]
            KTs = [persist.tile([128, 128], F16, name=f"kt{s}")
                   for s in range(TC)]
            Vs = [persist.tile([128, 128], F16, name=f"v{s}")
                  for s in range(TC)]
            OTs = [[persist.tile([128, 512], F16, name=f"ot{h}_{tb}")
                    for tb in range(TB)] for h in range(GH)]

            make_identity(nc, identF)
            # TRI[s, c] = 1 where c >= s (keep), else 0: causal keep-mask for
            # the diagonal 128x128 block.
            nc.gpsimd.memset(ONESC, 1.0)
            nc.gpsimd.memset(ONESR, 1.0)



            # ---- phase 1: projections + rope + transposes -------------
            with (
                tc.tile_pool(name="xtp", bufs=2) as xtp,
                tc.tile_pool(name="ropedst", bufs=2) as ropedst,
                tc.tile_pool(name="ropetmp", bufs=3) as ropetmp,
                tc.tile_pool(name="pq", bufs=2, space="PSUM") as pqp,
                tc.tile_pool(name="pkv", bufs=2, space="PSUM") as pkvp,
                tc.tile_pool(name="ptr", bufs=2, space="PSUM") as ptrp,
            ):
                def wchunk(c):
                    nc.sync.dma_start(
                        out=W[:, c * 4:(c + 1) * 4, :]
                        .rearrange("p k c -> p (k c)"),
                        in_=w[:, c * 4:(c + 1) * 4, :]
                        .rearrange("p k c -> p (k c)"))

                dsts = {}

                def emit_transpose(t, j):
                    # transpose roped q heads + k into QT / KT (fp16)
                    tp = ptrp.tile([128, 128], F16, tag="tp",
                                   padded_shape=[128, 1024],
                                   name=f"tp{t}_{j}")
                    nc.tensor.transpose(tp, dsts[t][:, j * 128:(j + 1) * 128],
                                        identF)
                    if j < GH:
                        nc.scalar.copy(
                            QTs[j][t // 4][:, (t % 4) * 128:(t % 4 + 1) * 128],
                            tp)
                    else:
                        nc.scalar.copy(KTs[t], tp)

                for t in range(TC):
                    xt_t = xtp.tile([128, KD, 128], F16)
                    if t == 0:
                        # interleave the first x^T chunk with the constants so
                        # the k-th projection's inputs land just in time
                        for c in range(4):
                            nc.sync.dma_start(
                                out=xt_t[:, c * 4:(c + 1) * 4, :],
                                in_=xt[0][:, c * 4:(c + 1) * 4, :]
                                .rearrange("p k c -> p (k c)"))
                            wchunk(c)
                            if c == 0:
                                nc.sync.dma_start(
                                    out=CS, in_=cs.rearrange("p k c -> p (k c)"))
                        nc.sync.dma_start(out=TRI, in_=tri[:, :])
                    else:
                        nc.sync.dma_start(out=xt_t,
                                          in_=xt[t].rearrange("p k c -> p (k c)"))
                    pq = pqp.tile([128, 512], F32)
                    pkv = pkvp.tile([128, 256], F32, padded_shape=[128, 512])
                    for k in range(KD):
                        lhs = xt_t[:, k, :]
                        nc.tensor.matmul(pq, lhs, W[:, k, 0:512],
                                         start=(k == 0), stop=(k == KD - 1))
                        nc.tensor.matmul(pkv, lhs, W[:, k, 512:768],
                                         start=(k == 0), stop=(k == KD - 1))
                        if t > 0 and 4 <= k < 9:
                            emit_transpose(t - 1, k - 4)
                    # rope (q: 4 heads batched as 3D on DVE; k: single head on Pool)
                    dst = ropedst.tile([128, 640], F16)
                    dst3 = dst.rearrange("p (h c) -> p h c", c=128)
                    pq3 = pq.rearrange("p (h c) -> p h c", c=128)
                    cosb = CS[:, t, None, 0:HALF].to_broadcast((128, GH, HALF))
                    sinb = CS[:, t, None, HALF:128].to_broadcast((128, GH, HALF))
                    q1, q2 = pq3[:, :, 0:HALF], pq3[:, :, HALF:128]
                    t1 = ropetmp.tile([128, GH, HALF], F32, tag="rt")
                    t2 = ropetmp.tile([128, GH, HALF], F32, tag="rt")
                    nc.vector.tensor_mul(t1, q1, cosb)
                    nc.vector.tensor_mul(t2, q2, sinb)
                    nc.vector.tensor_sub(dst3[:, 0:GH, 0:HALF], t1, t2)
                    t3 = ropetmp.tile([128, GH, HALF], F32, tag="rt")
                    t4 = ropetmp.tile([128, GH, HALF], F32, tag="rt")
                    nc.vector.tensor_mul(t3, q2, cosb)
                    nc.vector.tensor_mul(t4, q1, sinb)
                    nc.vector.tensor_add(dst3[:, 0:GH, HALF:128], t3, t4)
                    cos2, sin2 = CS[:, t, 0:HALF], CS[:, t, HALF:128]
                    k1, k2 = pkv[:, 0:HALF], pkv[:, HALF:128]
                    t5 = ropetmp.tile([128, HALF], F32, tag="rk")
                    t6 = ropetmp.tile([128, HALF], F32, tag="rk")
                    nc.vector.tensor_mul(t5, k1, cos2)
                    nc.vector.tensor_mul(t6, k2, sin2)
                    nc.vector.tensor_sub(dst[:, 512:576], t5, t6)
                    t7 = ropetmp.tile([128, HALF], F32, tag="rk")
                    t8 = ropetmp.tile([128, HALF], F32, tag="rk")
                    nc.vector.tensor_mul(t7, k2, cos2)
                    nc.vector.tensor_mul(t8, k1, sin2)
                    nc.vector.tensor_add(dst[:, 576:640], t7, t8)
                    nc.scalar.copy(Vs[t], pkv[:, 128:256])
                    dsts[t] = dst
                for j in range(5):
                    emit_transpose(TC - 1, j)

            # wo arrives while attention runs.
            nc.sync.dma_start(out=WO, in_=wo.rearrange("p h c -> p (h c)"))

            # ---- phase 2+3: attention with interleaved out-projection --
            with (
                tc.tile_pool(name="ptp", bufs=6) as ptp,
                tc.tile_pool(name="laccp", bufs=2) as laccp,
                tc.tile_pool(name="recp", bufs=2) as recp,
                tc.tile_pool(name="stagep", bufs=2) as stagep,
                tc.tile_pool(name="pst", bufs=2, space="PSUM") as pstp,
                tc.tile_pool(name="pot", bufs=2, space="PSUM") as potp,
                tc.tile_pool(name="pden", bufs=1, space="PSUM") as pdenp,
                tc.tile_pool(name="pbc", bufs=1, space="PSUM") as pbcp,
                tc.tile_pool(name="po", bufs=2, space="PSUM") as pop,
            ):
                def outproj_units(tb):
                    """Yield one emission step of out-projection for block tb."""
                    for i in range(4):
                        tq = tb * 4 + i
                        stage = stagep.tile([128, D_MODEL], F16, tag="stage",
                                            name=f"stage{tq}")
                        for n in range(4):
                            po = pop.tile([128, 512], F32, tag="po",
                                          name=f"po{tq}_{n}")
                            for h in range(GH):
                                yield (nc.tensor.matmul, po,
                                       OTs[h][tb][:, i * 128:(i + 1) * 128],
                                       WO[:, h, n * 512:(n + 1) * 512],
                                       dict(start=(h == 0), stop=(h == GH - 1)))
                            yield (nc.scalar.copy,
                                   stage[:, n * 512:(n + 1) * 512], po)
                        yield (nc.sync.dma_start,
                               dict(out=out[tq * 128:(tq + 1) * 128, :],
                                    in_=stage))

                def run_unit(u):
                    if isinstance(u[-1], dict) and len(u) == 2:
                        u[0](**u[1])
                    elif isinstance(u[-1], dict):
                        u[0](*u[1:-1], **u[-1])
                    else:
                        u[0](*u[1:])

                filler = None

                def fill(k):
                    nonlocal filler
                    if filler is None:
                        return
                    for _ in range(k):
                        u = next(filler, None)
                        if u is None:
                            filler = None
                            return
                        run_unit(u)

                pending = None
                pending_cross = False
                for tb in range(TB):
                    for h in range(GH):
                        nsc = 4 * (tb + 1)
                        # the tb-crossing normalize must land before the filler
                        # that reads OTs[3][tb-1] (consumed at sc==7)
                        flush_sc = min(5 if pending_cross else 7, nsc - 1)
                        ot = potp.tile([128, 512], F32, tag="ot",
                                       name=f"otp{h}_{tb}")
                        lacc = laccp.tile([128, 512], F16, tag="lacc",
                                          name=f"la{h}_{tb}")
                        for sc in range(nsc):
                            off = max(0, sc - 4 * tb) * 128
                            wdt = 512 - off
                            st = pstp.tile([128, 512], F32, tag="st",
                                           name=f"st{h}_{tb}_{sc}")
                            nc.tensor.matmul(st[:, 0:wdt], KTs[sc],
                                             QTs[h][tb][:, off:512],
                                             start=True, stop=True)
                            if sc == flush_sc and pending is not None:
                                pending()
                                pending = None
                            if sc >= 4:
                                fill(1)
                            pt = ptp.tile([128, 512], F16, tag="pt",
                                          name=f"pt{h}_{tb}_{sc}")
                            nc.scalar.activation(pt[:, 0:wdt], st[:, 0:wdt],
                                                 AF.Exp, scale=SCALE)
                            if sc >= 4 * tb:
                                nc.vector.tensor_mul(pt[:, 0:128],
                                                     pt[:, 0:128], TRI)
                            if sc == 0:
                                nc.vector.tensor_copy(lacc, pt)
                            else:
                                nc.vector.tensor_add(lacc[:, off:512],
                                                     lacc[:, off:512],
                                                     pt[:, 0:wdt])
                            nc.tensor.matmul(ot[:, off:512], Vs[sc],
                                             pt[:, 0:wdt],
                                             start=(sc == 0),
                                             stop=(sc == nsc - 1))
                        # keep the PE fed while DVE finishes the block's
                        # denominator adds
                        fill(4)
                        # transposed denominator: denT[q,0] = sum_s lacc[s,q]
                        # per q-chunk (~1-cycle matmuls), so the reciprocal
                        # runs on 128 partitions (~0.1us instead of 3.3us)
                        denT = pdenp.tile([128, 4], F32, tag="den",
                                          name=f"den{h}_{tb}")
                        for qc in range(4):
                            nc.tensor.matmul(denT[:, qc:qc + 1],
                                             lacc[:, qc * 128:(qc + 1) * 128],
                                             ONESC, start=True, stop=True)
                        recT = recp.tile([128, 4], F16, tag="recT",
                                         name=f"recT{h}_{tb}")
                        with nc.allow_low_precision("fp16 softmax denominators"):
                            nc.vector.reciprocal(recT, denT)
                        # scatter the reciprocal column back into row layout
                        # with a tiny SBUF->SBUF DMA (off both PE and DVE)
                        # linear scatter: rec stores q interleaved as p*4+c;
                        # the bc matmul's rhs AP undoes the permutation
                        rec = recp.tile([1, 512], F16, tag="rec",
                                        name=f"rec{h}_{tb}")
                        nc.sync.dma_start(out=rec, in_=recT)
                        # unnormalized copy frees the ot PSUM bank early
                        nc.vector.tensor_copy(OTs[h][tb], ot)

                        def fin(h=h, tb=tb, rec=rec):
                            bc = pbcp.tile([128, 512], F32, tag="bc",
                                           name=f"bc{h}_{tb}")
                            nc.tensor.matmul(
                                bc, ONESR,
                                rec.rearrange("a (p c) -> a c p", c=4),
                                start=True, stop=True)
                            nc.vector.tensor_mul(OTs[h][tb], OTs[h][tb], bc)
                        pending = fin
                        pending_cross = (h == GH - 1)
                        fill(2)
                    # drain leftover fillers of the previous block before
                    # switching to this block's out-projection
                    fill(10**6)
                    filler = outproj_units(tb)
                    if tb == TB - 1 and pending is not None:
                        fill(3)
                        pending()
                        pending = None
                # tail: the last block's out-projection
                fill(10**6)

    nc.compile()
    return nc


def _prep_core_inputs(x_b, wq, wk, wv, wo, cs_cat, g):
    wq_g = wq[:, g * 512:(g + 1) * 512]
    wk_g = wk[:, g * 128:(g + 1) * 128]
    wv_g = wv[:, g * 128:(g + 1) * 128]
    wqkv = np.concatenate([wq_g, wk_g, wv_g], axis=1)          # [D, 768]
    w_t = np.ascontiguousarray(wqkv.reshape(KD, 128, 768).transpose(1, 0, 2))
    wo_g = wo[g * 512:(g + 1) * 512, :]                         # [512, D]
    wo_t = np.ascontiguousarray(wo_g.reshape(GH, 128, D_MODEL).transpose(1, 0, 2))
    xt = np.ascontiguousarray(
        x_b.reshape(TC, 128, KD, 128).transpose(0, 3, 2, 1))    # [tc,j,k,i]
    tri = (np.arange(128)[None, :] >= np.arange(128)[:, None])
    return {
        "xt": xt.astype(np.float16),
        "w": w_t.astype(np.float16),
        "wo": wo_t.astype(np.float16),
        "cs": cs_cat.astype(np.float32),
        "tri": tri.astype(np.float16),
    }


def _prep_all(x, wq, wk, wv, wo, cos, sin):
    cs = np.concatenate([cos, sin], axis=1)                     # [T, 128]
    cs_t = np.ascontiguousarray(
        cs.reshape(TC, 128, 128).transpose(1, 0, 2)).astype(np.float32)
    in_maps = []
    for i in range(8):
        b, g = i // 4, i % 4
        in_maps.append(_prep_core_inputs(x[b], wq, wk, wv, wo, cs_t, g))
    return in_maps


def kernel(x, wq, wk, wv, wo, cos, sin):
    x = np.asarray(x, np.float32)
    wq = np.asarray(wq, np.float32)
    wk = np.asarray(wk, np.float32)
    wv = np.asarray(wv, np.float32)
    wo = np.asarray(wo, np.float32)
    cos = np.asarray(cos, np.float32)
    sin = np.asarray(sin, np.float32)

    nc = build_nc()
    in_maps = _prep_all(x, wq, wk, wv, wo, cos, sin)
    res = run_bass_kernel_spmd(nc, in_maps, list(range(8)))
    outs = [np.asarray(res.results[i]["out"], np.float32) for i in range(8)]
    full = np.empty((B, T, D_MODEL), np.float32)
    for b in range(B):
        full[b] = outs[4 * b] + outs[4 * b + 1] + outs[4 * b + 2] + outs[4 * b + 3]
    return full
